# revision 16
# baseline (speedup 1.0000x reference)
"""Multi-head attention (B=2, N=4096, D=768, H=8) on 8 trn2 NeuronCores.

Sharding: core c handles batch b = c//4 and head-pair hp = c%4 (heads 2hp,
2hp+1).  Each core computes qkv projection for its 2 heads plus full
4096x4096 attention for them; no cross-core communication.

Device-side layout (per core):
  xT    [768, 4096] fp16   x[b] transposed (host-prepped)
  wqk   [768, 384]  fp16   [Wq_h0*scale | Wq_h1*scale | Wk_h0 | Wk_h1]
  bqk   [96, 4]     fp32   matching biases as per-partition columns
  wv    [768, 194]  fp16   [Wv_h0 | 0 | Wv_h1 | 0]
  wvaug [1, 194]    fp16   [bv_h0 | 1 | bv_h1 | 1]  (ones row of aug x)
  out   [2, 4096, 96] fp32 per-head attention output

Algorithm: qT/kT = W.T @ xT in [dh, tok] layout; V in [tok, dh(+1)] layout
via xT-stationary matmuls (ones column for softmax row sums).  Scores are
computed transposed S^T[m, n] = kT_tile.T @ qT (contract dh=96), exp'ed on
ScalarE straight out of PSUM (no max subtraction: |scores| <~ 2.5 for this
distribution), and PV accumulates out[n, 97] over 32 key tiles with the
exp tile as the stationary operand.  Row 96 holds the softmax denominator;
a reciprocal + per-partition multiply normalizes.
"""

import sys

for _p in ("/opt/trn_rl_repo",):
    if _p not in sys.path:
        sys.path.insert(0, _p)

import numpy as np

B = 2
N = 4096
DIM = 768
H = 8
DH = 96
SCALE = DIM ** -0.5
NCORES = 8
VW = 2 * DH + 2  # 194: [v_h0 | ones | v_h1 | ones]
NT = N // 128    # 32 token tiles
NBLK = N // 512  # 8 blocks of 512
DT = DIM // 128  # 6 contraction tiles

_CACHE = {}


def build_program(loop_iters=1, variant="full"):
    import concourse.tile as tile
    from concourse import bacc, mybir

    F16 = mybir.dt.float16
    F32 = mybir.dt.float32
    Exp = mybir.ActivationFunctionType.Exp

    nc = bacc.Bacc("TRN2", target_bir_lowering=False, debug=False)
    xT_h = nc.declare_dram_parameter("xT", [DIM, N], F16, isOutput=False)
    wqk_h = nc.declare_dram_parameter("wqk", [DIM, 4 * DH], F16, isOutput=False)
    bqk_h = nc.declare_dram_parameter("bqk", [DH, 4], F32, isOutput=False)
    wv_h = nc.declare_dram_parameter("wv", [DIM, VW], F16, isOutput=False)
    wvaug_h = nc.declare_dram_parameter("wvaug", [1, VW], F16, isOutput=False)
    out_h = nc.declare_dram_parameter("out", [2, N, DH], F32, isOutput=True)

    xT, wqk, bqk = xT_h.ap(), wqk_h.ap(), bqk_h.ap()
    wv, wvaug, out = wv_h.ap(), wvaug_h.ap(), out_h.ap()

    # m-tile groups for the scores/exp pipeline: 3 psum banks double-buffered
    groups = []
    m0 = 0
    while m0 < NT:
        gsz = min(3, NT - m0)
        groups.append((m0, gsz))
        m0 += gsz

    with tile.TileContext(nc) as tc:
        with (
            tc.tile_pool(name="const", bufs=1) as const,
            tc.tile_pool(name="work", bufs=3) as work,
            tc.tile_pool(name="pp", bufs=2, space="PSUM") as pp,
        ):
            # --- persistent SBUF tensors ---
            xt_sb = [
                const.tile([128, N], F16, name=f"xt{d}", tag=f"xt{d}")
                for d in range(DT)
            ]
            wqk_sb = [
                const.tile([128, 4 * DH], F16, name=f"wqksb{d}", tag=f"wqksb{d}")
                for d in range(DT)
            ]
            wv_sb = [
                const.tile([128, VW], F16, name=f"wvsb{d}", tag=f"wvsb{d}")
                for d in range(DT)
            ]
            wvaug_sb = const.tile([1, VW], F16, name="wvaug_sb")
            bqk_sb = const.tile([DH, 4], F32, name="bqk_sb")
            ones_sb = const.tile([1, N], F16, name="ones_sb")
            zrow_sb = const.tile([1, 512], F16, name="zrow_sb")
            qkT_sb = [
                const.tile([DH, N], F16, name=f"qkT{j}", tag=f"qkT{j}")
                for j in range(4)
            ]
            v_sb = const.tile([128, NT * VW], F16, name="v_sb")

            nc.sync.dma_start(out=bqk_sb, in_=bqk)
            nc.sync.dma_start(out=wvaug_sb, in_=wvaug)
            for d in range(DT):
                nc.sync.dma_start(out=wqk_sb[d], in_=wqk[d * 128:(d + 1) * 128, :])
                nc.sync.dma_start(out=wv_sb[d], in_=wv[d * 128:(d + 1) * 128, :])
            # xT arrives in column chunks, in the order the first attention
            # window consumes them.
            for blk in range(NBLK):
                for d in range(DT):
                    nc.sync.dma_start(
                        out=xt_sb[d][:, blk * 512:(blk + 1) * 512],
                        in_=xT[d * 128:(d + 1) * 128, blk * 512:(blk + 1) * 512],
                    )
            nc.vector.memset(ones_sb, 1.0)
            nc.vector.memset(zrow_sb, 0.0)

            qk_done = set()
            v_done = set()

            def ensure_qk(j, blk):
                # qkT_sb[j][:, blk] = (wqk[:, j] block).T @ xT[:, blk] + bias_j
                if (j, blk) in qk_done:
                    return
                qk_done.add((j, blk))
                pt = pp.tile([DH, 512], F32, tag="pva", name="pt", bufs=2)
                for d in range(DT):
                    nc.tensor.matmul(
                        pt,
                        lhsT=wqk_sb[d][:, j * DH:(j + 1) * DH],
                        rhs=xt_sb[d][:, blk * 512:(blk + 1) * 512],
                        start=(d == 0),
                        stop=(d == DT - 1),
                    )
                nc.vector.tensor_scalar_add(
                    out=qkT_sb[j][:, blk * 512:(blk + 1) * 512],
                    in0=pt,
                    scalar1=bqk_sb[:, j:j + 1],
                )

            def ensure_v(t):
                if t in v_done:
                    return
                v_done.add(t)
                pv = pp.tile([128, VW], F32, tag="pva", name="pv", bufs=2)
                for d in range(DT):
                    nc.tensor.matmul(
                        pv,
                        lhsT=xt_sb[d][:, t * 128:(t + 1) * 128],
                        rhs=wv_sb[d],
                        start=(d == 0),
                        stop=False,
                    )
                nc.tensor.matmul(
                    pv,
                    lhsT=ones_sb[:, t * 128:(t + 1) * 128],
                    rhs=wvaug_sb,
                    start=False,
                    stop=True,
                )
                nc.vector.tensor_copy(out=v_sb[:, t * VW:(t + 1) * VW], in_=pv)

            # filler: projection units to slip into PE slack inside the
            # ACT-bound attention stream, ordered by deadline.
            filler = []
            for b in range(1, NBLK):
                filler.append((0, b))       # q_h0 blk b: before window (0, b)
                filler.append((3, b - 1))   # k_h1: all before head 1
            filler.append((3, NBLK - 1))
            for b in range(NBLK):
                filler.append((1, b))       # q_h1 blk b: before window (1, b)
            fill_state = {"i": 0, "tick": 0}

            def pop_filler():
                fill_state["tick"] += 1
                if fill_state["tick"] % 4 == 0 and fill_state["i"] < len(filler):
                    j, b = filler[fill_state["i"]]
                    fill_state["i"] += 1
                    ensure_qk(j, b)

            def attn_nw(h, nw):
                qT = qkT_sb[h]
                kT = qkT_sb[2 + h]
                ensure_qk(h, nw)
                pva = pp.tile([128, 512], F32, tag="pva", name="pva", bufs=2)
                # Zero the accumulator bank with a K=1 matmul so every PV
                # matmul can be a plain accumulate (order-independent).
                nc.tensor.matmul(
                    pva,
                    lhsT=ones_sb[:, :128],
                    rhs=zrow_sb,
                    start=True,
                    stop=True,
                )
                def emit_pv(g0, gsz, ex):
                    if variant == "nopv":
                        return
                    for i in range(gsz):
                        mt = g0 + i
                        for ns in range(4):
                            nc.tensor.matmul(
                                pva[:, ns * 97:ns * 97 + 97],
                                lhsT=ex[:, i * 512 + ns * 128:i * 512 + (ns + 1) * 128],
                                rhs=v_sb[:, mt * VW + h * 97:mt * VW + h * 97 + 97],
                                start=False,
                                stop=(mt == NT - 1 and ns == 3),
                                skip_group_check=True,
                            )

                pending = None  # software pipeline: PV(g-1) after scores(g)
                for (g0, gsz) in groups:
                    for b in range((g0 + gsz - 1) * 128 // 512 + 1):
                        ensure_qk(2 + h, b)
                    for t in range(g0, g0 + gsz):
                        ensure_v(t)
                    sc = pp.tile([128, 512 * gsz], F32, tag="sc", name="sc")
                    for i in range(gsz):
                        mt = g0 + i
                        nc.tensor.matmul(
                            sc[:, i * 512:(i + 1) * 512],
                            lhsT=kT[:, mt * 128:(mt + 1) * 128],
                            rhs=qT[:, nw * 512:(nw + 1) * 512],
                            start=True,
                            stop=True,
                        )
                    ex = work.tile([128, 512 * gsz], F16, tag="ex", name="ex", bufs=4)
                    if variant == "noexp":
                        # timing ablation: near-free ACT op keeps deps intact
                        nc.scalar.activation(out=ex[:, :8], in_=sc[:, :8], func=Exp)
                    else:
                        nc.scalar.activation(out=ex, in_=sc, func=Exp)
                    if pending is not None:
                        emit_pv(*pending)
                        pop_filler()
                    pending = (g0, gsz, ex)
                emit_pv(*pending)
                pop_filler()
                rec = work.tile([128, 4], F32, tag="rec", name="rec", bufs=2)
                nc.vector.reciprocal(
                    out=rec,
                    in_=pva[:, :4 * 97].rearrange("p (a b) -> p a b", b=97)[:, :, 96],
                )
                ob = work.tile([128, 4 * DH], F32, tag="ob", name="ob")
                for ns in range(4):
                    nc.vector.tensor_scalar_mul(
                        out=ob[:, ns * DH:(ns + 1) * DH],
                        in0=pva[:, ns * 97:ns * 97 + DH],
                        scalar1=rec[:, ns:ns + 1],
                    )
                nc.sync.dma_start(
                    out=out[h, nw * 512:(nw + 1) * 512, :].rearrange(
                        "(a p) c -> p a c", p=128
                    ),
                    in_=ob.rearrange("p (a c) -> p a c", c=DH),
                )

            # Emission order tuned for overlap: head-0 q/k projection and V
            # first, then attention for head 0 with head-1 projections
            # slipped in between the first windows.
            def body(_i=None):
                qk_done.clear()
                v_done.clear()
                fill_state["i"] = 0
                fill_state["tick"] = 0
                for h in range(2):
                    for nw in range(NBLK):
                        attn_nw(h, nw)
                # backstop: anything the filler didn't reach
                for j, b in filler:
                    ensure_qk(j, b)

            if loop_iters == 1:
                body()
            else:
                with tc.For_i(0, loop_iters, 1) as _i:
                    body(_i)

    nc.compile()
    return nc


def get_program(loop_iters=1, variant="full"):
    key = ("nc", loop_iters, variant)
    if key not in _CACHE:
        _CACHE[key] = build_program(loop_iters, variant)
    return _CACHE[key]


def make_in_maps(x, W_qkv, b_qkv):
    x = np.asarray(x, np.float32)
    W = np.asarray(W_qkv, np.float32)
    b = np.asarray(b_qkv, np.float32)
    Wq, Wk, Wv = W[:, :DIM], W[:, DIM:2 * DIM], W[:, 2 * DIM:]
    bq, bk, bv = b[:DIM], b[DIM:2 * DIM], b[2 * DIM:]

    in_maps = []
    for c in range(NCORES):
        bb, hp = divmod(c, 4)
        h0 = 2 * hp
        s = slice(h0 * DH, (h0 + 1) * DH)
        s1 = slice((h0 + 1) * DH, (h0 + 2) * DH)
        xT = np.ascontiguousarray(x[bb].T).astype(np.float16)
        wqk = np.concatenate(
            [Wq[:, s] * SCALE, Wq[:, s1] * SCALE, Wk[:, s], Wk[:, s1]], axis=1
        ).astype(np.float16)
        bqk = np.stack(
            [bq[s] * SCALE, bq[s1] * SCALE, bk[s], bk[s1]], axis=1
        ).astype(np.float32)
        wv = np.zeros((DIM, VW), np.float16)
        wv[:, 0:DH] = Wv[:, s].astype(np.float16)
        wv[:, DH + 1:2 * DH + 1] = Wv[:, s1].astype(np.float16)
        wvaug = np.zeros((1, VW), np.float16)
        wvaug[0, 0:DH] = bv[s].astype(np.float16)
        wvaug[0, DH] = 1.0
        wvaug[0, DH + 1:2 * DH + 1] = bv[s1].astype(np.float16)
        wvaug[0, 2 * DH + 1] = 1.0
        in_maps.append(
            {"xT": xT, "wqk": wqk, "bqk": bqk, "wv": wv, "wvaug": wvaug}
        )
    return in_maps


def gather_out(results):
    out = np.empty((B, N, DIM), np.float32)
    for c in range(NCORES):
        bb, hp = divmod(c, 4)
        o = np.asarray(results[c]["out"], np.float32)  # [2, N, DH]
        out[bb, :, (2 * hp) * DH:(2 * hp + 1) * DH] = o[0]
        out[bb, :, (2 * hp + 1) * DH:(2 * hp + 2) * DH] = o[1]
    return out


def run(x, W_qkv, b_qkv, trace=False, **kw):
    from concourse.bass_utils import run_bass_kernel_spmd

    nc = get_program()
    in_maps = make_in_maps(x, W_qkv, b_qkv)
    res = run_bass_kernel_spmd(nc, in_maps, list(range(NCORES)), trace=trace, **kw)
    return gather_out(res.results), res


def kernel(x, W_qkv, b_qkv):
    out, _ = run(x, W_qkv, b_qkv)
    return out


# revision 18
# speedup vs baseline: 1.0012x; 1.0012x over previous
"""Multi-head attention (B=2, N=4096, D=768, H=8) on 8 trn2 NeuronCores.

Sharding: core c handles batch b = c//4 and head-pair hp = c%4 (heads 2hp,
2hp+1).  Each core computes qkv projection for its 2 heads plus full
4096x4096 attention for them; no cross-core communication.

Device-side layout (per core):
  xT    [768, 4096] fp16   x[b] transposed (host-prepped)
  wqk   [768, 384]  fp16   [Wq_h0*scale | Wq_h1*scale | Wk_h0 | Wk_h1]
  bqk   [96, 4]     fp32   matching biases as per-partition columns
  wv    [768, 194]  fp16   [Wv_h0 | 0 | Wv_h1 | 0]
  wvaug [1, 194]    fp16   [bv_h0 | 1 | bv_h1 | 1]  (ones row of aug x)
  out   [2, 4096, 96] fp32 per-head attention output

Algorithm: qT/kT = W.T @ xT in [dh, tok] layout; V in [tok, dh(+1)] layout
via xT-stationary matmuls (ones column for softmax row sums).  Scores are
computed transposed S^T[m, n] = kT_tile.T @ qT (contract dh=96), exp'ed on
ScalarE straight out of PSUM (no max subtraction: |scores| <~ 2.5 for this
distribution), and PV accumulates out[n, 97] over 32 key tiles with the
exp tile as the stationary operand.  Row 96 holds the softmax denominator;
a reciprocal + per-partition multiply normalizes.
"""

import sys

for _p in ("/opt/trn_rl_repo",):
    if _p not in sys.path:
        sys.path.insert(0, _p)

import numpy as np

B = 2
N = 4096
DIM = 768
H = 8
DH = 96
SCALE = DIM ** -0.5
NCORES = 8
VW = 256  # per m-tile V layout: [v_h0 | 1 | 0pad][v_h1 | 1 | 0pad], 128 each
NT = N // 128    # 32 token tiles
NBLK = N // 512  # 8 blocks of 512
DT = DIM // 128  # 6 contraction tiles

_CACHE = {}


def build_program(loop_iters=1, variant="full"):
    import concourse.tile as tile
    from concourse import bacc, mybir

    F16 = mybir.dt.float16
    F32 = mybir.dt.float32
    Exp = mybir.ActivationFunctionType.Exp

    nc = bacc.Bacc("TRN2", target_bir_lowering=False, debug=False)
    xT_h = nc.declare_dram_parameter("xT", [DIM, N], F16, isOutput=False)
    wqk_h = nc.declare_dram_parameter("wqk", [DIM, 4 * DH], F16, isOutput=False)
    bqk_h = nc.declare_dram_parameter("bqk", [DH, 4], F32, isOutput=False)
    wv_h = nc.declare_dram_parameter("wv", [DIM, VW], F16, isOutput=False)
    wvaug_h = nc.declare_dram_parameter("wvaug", [1, VW], F16, isOutput=False)
    ident_h = nc.declare_dram_parameter("ident", [128, 128], F16, isOutput=False)
    out_h = nc.declare_dram_parameter("out", [2, N, DH], F32, isOutput=True)

    xT, wqk, bqk = xT_h.ap(), wqk_h.ap(), bqk_h.ap()
    wv, wvaug, out = wv_h.ap(), wvaug_h.ap(), out_h.ap()
    ident = ident_h.ap()

    # m-tile groups for the scores/exp pipeline: 3 psum banks double-buffered
    groups = []
    m0 = 0
    while m0 < NT:
        gsz = min(3, NT - m0)
        groups.append((m0, gsz))
        m0 += gsz

    with tile.TileContext(nc) as tc:
        with (
            tc.tile_pool(name="const", bufs=1) as const,
            tc.tile_pool(name="work", bufs=3) as work,
            tc.tile_pool(name="pp", bufs=2, space="PSUM") as pp,
        ):
            # --- persistent SBUF tensors ---
            xt_sb = [
                const.tile([128, N], F16, name=f"xt{d}", tag=f"xt{d}")
                for d in range(DT)
            ]
            wqk_sb = [
                const.tile([128, 4 * DH], F16, name=f"wqksb{d}", tag=f"wqksb{d}")
                for d in range(DT)
            ]
            wv_sb = [
                const.tile([128, VW], F16, name=f"wvsb{d}", tag=f"wvsb{d}")
                for d in range(DT)
            ]
            wvaug_sb = const.tile([1, VW], F16, name="wvaug_sb")
            bqk_sb = const.tile([DH, 4], F32, name="bqk_sb")
            ones_sb = const.tile([1, N], F16, name="ones_sb")
            ident_sb = const.tile([128, 128], F16, name="ident_sb")
            qkT_sb = [
                const.tile([DH, N], F16, name=f"qkT{j}", tag=f"qkT{j}")
                for j in range(4)
            ]
            v_sb = const.tile([128, NT * VW], F16, name="v_sb")

            nc.sync.dma_start(out=bqk_sb, in_=bqk)
            nc.sync.dma_start(out=ident_sb, in_=ident)
            nc.sync.dma_start(out=wvaug_sb, in_=wvaug)
            for d in range(DT):
                nc.sync.dma_start(out=wqk_sb[d], in_=wqk[d * 128:(d + 1) * 128, :])
                nc.sync.dma_start(out=wv_sb[d], in_=wv[d * 128:(d + 1) * 128, :])
            # xT arrives in column chunks, in the order the first attention
            # window consumes them.
            for blk in range(NBLK):
                for d in range(DT):
                    nc.sync.dma_start(
                        out=xt_sb[d][:, blk * 512:(blk + 1) * 512],
                        in_=xT[d * 128:(d + 1) * 128, blk * 512:(blk + 1) * 512],
                    )
            nc.vector.memset(ones_sb, 1.0)

            qk_done = set()
            v_done = set()

            def ensure_qk(j, blk):
                # qkT_sb[j][:, blk] = (wqk[:, j] block).T @ xT[:, blk] + bias_j
                if (j, blk) in qk_done:
                    return
                qk_done.add((j, blk))
                pt = pp.tile([DH, 512], F32, tag="pj", name="pt", bufs=1)
                for d in range(DT):
                    nc.tensor.matmul(
                        pt,
                        lhsT=wqk_sb[d][:, j * DH:(j + 1) * DH],
                        rhs=xt_sb[d][:, blk * 512:(blk + 1) * 512],
                        start=(d == 0),
                        stop=(d == DT - 1),
                    )
                nc.vector.tensor_scalar_add(
                    out=qkT_sb[j][:, blk * 512:(blk + 1) * 512],
                    in0=pt,
                    scalar1=bqk_sb[:, j:j + 1],
                )

            def ensure_v(t):
                if t in v_done:
                    return
                v_done.add(t)
                pv = pp.tile([128, VW], F32, tag="pj", name="pv", bufs=1)
                for d in range(DT):
                    nc.tensor.matmul(
                        pv,
                        lhsT=xt_sb[d][:, t * 128:(t + 1) * 128],
                        rhs=wv_sb[d],
                        start=(d == 0),
                        stop=False,
                    )
                nc.tensor.matmul(
                    pv,
                    lhsT=ones_sb[:, t * 128:(t + 1) * 128],
                    rhs=wvaug_sb,
                    start=False,
                    stop=True,
                )
                nc.vector.tensor_copy(out=v_sb[:, t * VW:(t + 1) * VW], in_=pv)

            # filler: projection units to slip into PE slack inside the
            # ACT-bound attention stream, ordered by deadline.
            filler = []
            for b in range(1, NBLK):
                filler.append((0, b))       # q_h0 blk b: before window (0, b)
                filler.append((3, b - 1))   # k_h1: all before head 1
            filler.append((3, NBLK - 1))
            for b in range(NBLK):
                filler.append((1, b))       # q_h1 blk b: before window (1, b)
            fill_state = {"i": 0, "tick": 0}

            def pop_filler():
                fill_state["tick"] += 1
                if fill_state["tick"] % 4 == 0 and fill_state["i"] < len(filler):
                    j, b = filler[fill_state["i"]]
                    fill_state["i"] += 1
                    ensure_qk(j, b)

            def attn_nw(h, nw):
                qT = qkT_sb[h]
                kT = qkT_sb[2 + h]
                ensure_qk(h, nw)
                pva = pp.tile([128, 512], F32, tag="pva", name="pva", bufs=1)

                def emit_pv(g0, gsz, ex):
                    if variant == "nopv":
                        return
                    for i in range(gsz):
                        mt = g0 + i
                        nc.tensor.matmul(
                            pva,
                            lhsT=v_sb[:, mt * VW + h * 128:mt * VW + (h + 1) * 128],
                            rhs=ex[:, i * 512:(i + 1) * 512],
                            start=(mt == 0),
                            stop=(mt == NT - 1),
                        )

                pending = None  # software pipeline: PV(g-1) after scores(g)
                for (g0, gsz) in groups:
                    for b in range((g0 + gsz - 1) * 128 // 512 + 1):
                        ensure_qk(2 + h, b)
                    for t in range(g0, g0 + gsz):
                        ensure_v(t)
                    sc = pp.tile([128, 512 * gsz], F32, tag="sc", name="sc")
                    for i in range(gsz):
                        mt = g0 + i
                        nc.tensor.matmul(
                            sc[:, i * 512:(i + 1) * 512],
                            lhsT=kT[:, mt * 128:(mt + 1) * 128],
                            rhs=qT[:, nw * 512:(nw + 1) * 512],
                            start=True,
                            stop=True,
                        )
                    ex = work.tile([128, 512 * gsz], F16, tag="ex", name="ex", bufs=4)
                    if variant == "noexp":
                        # timing ablation: near-free ACT op keeps deps intact
                        nc.scalar.activation(out=ex[:, :8], in_=sc[:, :8], func=Exp)
                    else:
                        nc.scalar.activation(out=ex, in_=sc, func=Exp)
                    if pending is not None:
                        emit_pv(*pending)
                        pop_filler()
                    pending = (g0, gsz, ex)
                emit_pv(*pending)
                pop_filler()
                # epilogue: out^T [d(+sum), n] -> fp16 sbuf -> PE transpose to
                # [n, d] -> normalize by row 96 (the softmax denominator)
                ot = work.tile([128, 512], F16, tag="ot", name="ot")
                nc.vector.tensor_copy(out=ot, in_=pva)
                tp = pp.tile([128, 512], F16, tag="pj", name="tp", bufs=1)
                for k in range(4):
                    nc.tensor.matmul(
                        tp[:, k * 128:(k + 1) * 128],
                        lhsT=ot[:, k * 128:(k + 1) * 128],
                        rhs=ident_sb,
                        is_transpose=True,
                        start=(k == 0),
                        stop=(k == 3),
                    )
                rec = work.tile([128, 4], F32, tag="rec", name="rec", bufs=2)
                nc.vector.reciprocal(
                    out=rec,
                    in_=tp.rearrange("p (a b) -> p a b", b=128)[:, :, DH],
                )
                ob = work.tile([128, 4 * DH], F32, tag="ob", name="ob")
                for ns in range(4):
                    nc.vector.tensor_scalar_mul(
                        out=ob[:, ns * DH:(ns + 1) * DH],
                        in0=tp[:, ns * 128:ns * 128 + DH],
                        scalar1=rec[:, ns:ns + 1],
                    )
                nc.sync.dma_start(
                    out=out[h, nw * 512:(nw + 1) * 512, :].rearrange(
                        "(a p) c -> p a c", p=128
                    ),
                    in_=ob.rearrange("p (a c) -> p a c", c=DH),
                )

            # Emission order tuned for overlap: head-0 q/k projection and V
            # first, then attention for head 0 with head-1 projections
            # slipped in between the first windows.
            def body(_i=None):
                qk_done.clear()
                v_done.clear()
                fill_state["i"] = 0
                fill_state["tick"] = 0
                for h in range(2):
                    for nw in range(NBLK):
                        attn_nw(h, nw)
                # backstop: anything the filler didn't reach
                for j, b in filler:
                    ensure_qk(j, b)

            if loop_iters == 1:
                body()
            else:
                with tc.For_i(0, loop_iters, 1) as _i:
                    body(_i)

    nc.compile()
    return nc


def get_program(loop_iters=1, variant="full"):
    key = ("nc", loop_iters, variant)
    if key not in _CACHE:
        _CACHE[key] = build_program(loop_iters, variant)
    return _CACHE[key]


def make_in_maps(x, W_qkv, b_qkv):
    x = np.asarray(x, np.float32)
    W = np.asarray(W_qkv, np.float32)
    b = np.asarray(b_qkv, np.float32)
    Wq, Wk, Wv = W[:, :DIM], W[:, DIM:2 * DIM], W[:, 2 * DIM:]
    bq, bk, bv = b[:DIM], b[DIM:2 * DIM], b[2 * DIM:]

    in_maps = []
    for c in range(NCORES):
        bb, hp = divmod(c, 4)
        h0 = 2 * hp
        s = slice(h0 * DH, (h0 + 1) * DH)
        s1 = slice((h0 + 1) * DH, (h0 + 2) * DH)
        xT = np.ascontiguousarray(x[bb].T).astype(np.float16)
        wqk = np.concatenate(
            [Wq[:, s] * SCALE, Wq[:, s1] * SCALE, Wk[:, s], Wk[:, s1]], axis=1
        ).astype(np.float16)
        bqk = np.stack(
            [bq[s] * SCALE, bq[s1] * SCALE, bk[s], bk[s1]], axis=1
        ).astype(np.float32)
        wv = np.zeros((DIM, VW), np.float16)
        wv[:, 0:DH] = Wv[:, s].astype(np.float16)
        wv[:, 128:128 + DH] = Wv[:, s1].astype(np.float16)
        wvaug = np.zeros((1, VW), np.float16)
        wvaug[0, 0:DH] = bv[s].astype(np.float16)
        wvaug[0, DH] = 1.0
        wvaug[0, 128:128 + DH] = bv[s1].astype(np.float16)
        wvaug[0, 128 + DH] = 1.0
        in_maps.append(
            {"xT": xT, "wqk": wqk, "bqk": bqk, "wv": wv, "wvaug": wvaug,
             "ident": np.eye(128, dtype=np.float16)}
        )
    return in_maps


def gather_out(results):
    out = np.empty((B, N, DIM), np.float32)
    for c in range(NCORES):
        bb, hp = divmod(c, 4)
        o = np.asarray(results[c]["out"], np.float32)  # [2, N, DH]
        out[bb, :, (2 * hp) * DH:(2 * hp + 1) * DH] = o[0]
        out[bb, :, (2 * hp + 1) * DH:(2 * hp + 2) * DH] = o[1]
    return out


def run(x, W_qkv, b_qkv, trace=False, **kw):
    from concourse.bass_utils import run_bass_kernel_spmd

    nc = get_program()
    in_maps = make_in_maps(x, W_qkv, b_qkv)
    res = run_bass_kernel_spmd(nc, in_maps, list(range(NCORES)), trace=trace, **kw)
    return gather_out(res.results), res


def kernel(x, W_qkv, b_qkv):
    out, _ = run(x, W_qkv, b_qkv)
    return out


# revision 19
# speedup vs baseline: 1.0990x; 1.0977x over previous
"""Multi-head attention (B=2, N=4096, D=768, H=8) on 8 trn2 NeuronCores.

Sharding: core c handles batch b = c//4 and head-pair hp = c%4 (heads 2hp,
2hp+1).  Each core computes qkv projection for its 2 heads plus full
4096x4096 attention for them; no cross-core communication.

Device-side layout (per core):
  xT    [768, 4096] fp16   x[b] transposed (host-prepped)
  wqk   [768, 384]  fp16   [Wq_h0*scale | Wq_h1*scale | Wk_h0 | Wk_h1]
  bqk   [96, 4]     fp32   matching biases as per-partition columns
  wv    [768, 194]  fp16   [Wv_h0 | 0 | Wv_h1 | 0]
  wvaug [1, 194]    fp16   [bv_h0 | 1 | bv_h1 | 1]  (ones row of aug x)
  out   [2, 4096, 96] fp32 per-head attention output

Algorithm: qT/kT = W.T @ xT in [dh, tok] layout; V in [tok, dh(+1)] layout
via xT-stationary matmuls (ones column for softmax row sums).  Scores are
computed transposed S^T[m, n] = kT_tile.T @ qT (contract dh=96), exp'ed on
ScalarE straight out of PSUM (no max subtraction: |scores| <~ 2.5 for this
distribution), and PV accumulates out[n, 97] over 32 key tiles with the
exp tile as the stationary operand.  Row 96 holds the softmax denominator;
a reciprocal + per-partition multiply normalizes.
"""

import sys

for _p in ("/opt/trn_rl_repo",):
    if _p not in sys.path:
        sys.path.insert(0, _p)

import numpy as np

B = 2
N = 4096
DIM = 768
H = 8
DH = 96
SCALE = DIM ** -0.5
NCORES = 8
VW = 2 * DH + 2  # 194: [v_h0 | ones | v_h1 | ones]
NT = N // 128    # 32 token tiles
NBLK = N // 512  # 8 blocks of 512
DT = DIM // 128  # 6 contraction tiles

_CACHE = {}


def build_program(loop_iters=1, variant="full"):
    import concourse.tile as tile
    from concourse import bacc, mybir

    F16 = mybir.dt.float16
    F32 = mybir.dt.float32
    Exp = mybir.ActivationFunctionType.Exp

    nc = bacc.Bacc("TRN2", target_bir_lowering=False, debug=False)
    xT_h = nc.declare_dram_parameter("xT", [DIM, N], F16, isOutput=False)
    wqk_h = nc.declare_dram_parameter("wqk", [DIM, 4 * DH], F16, isOutput=False)
    bqk_h = nc.declare_dram_parameter("bqk", [DH, 4], F32, isOutput=False)
    wv_h = nc.declare_dram_parameter("wv", [DIM, VW], F16, isOutput=False)
    wvaug_h = nc.declare_dram_parameter("wvaug", [1, VW], F16, isOutput=False)
    out_h = nc.declare_dram_parameter("out", [2, N, DH], F32, isOutput=True)

    xT, wqk, bqk = xT_h.ap(), wqk_h.ap(), bqk_h.ap()
    wv, wvaug, out = wv_h.ap(), wvaug_h.ap(), out_h.ap()

    # m-tile groups for the scores/exp pipeline: 3 psum banks double-buffered
    groups = []
    m0 = 0
    while m0 < NT:
        gsz = min(3, NT - m0)
        groups.append((m0, gsz))
        m0 += gsz

    with tile.TileContext(nc) as tc:
        with (
            tc.tile_pool(name="const", bufs=1) as const,
            tc.tile_pool(name="work", bufs=3) as work,
            tc.tile_pool(name="pp", bufs=2, space="PSUM") as pp,
        ):
            # --- persistent SBUF tensors ---
            xt_sb = [
                const.tile([128, N], F16, name=f"xt{d}", tag=f"xt{d}")
                for d in range(DT)
            ]
            wqk_sb = [
                const.tile([128, 4 * DH], F16, name=f"wqksb{d}", tag=f"wqksb{d}")
                for d in range(DT)
            ]
            wv_sb = [
                const.tile([128, VW], F16, name=f"wvsb{d}", tag=f"wvsb{d}")
                for d in range(DT)
            ]
            wvaug_sb = const.tile([1, VW], F16, name="wvaug_sb")
            bqk_sb = const.tile([DH, 4], F32, name="bqk_sb")
            ones_sb = const.tile([1, N], F16, name="ones_sb")
            zrow_sb = const.tile([1, 512], F16, name="zrow_sb")
            qkT_sb = [
                const.tile([DH, N], F16, name=f"qkT{j}", tag=f"qkT{j}")
                for j in range(4)
            ]
            v_sb = const.tile([128, NT * VW], F16, name="v_sb")

            nc.sync.dma_start(out=bqk_sb, in_=bqk)
            nc.sync.dma_start(out=wvaug_sb, in_=wvaug)
            for d in range(DT):
                nc.sync.dma_start(out=wqk_sb[d], in_=wqk[d * 128:(d + 1) * 128, :])
                nc.sync.dma_start(out=wv_sb[d], in_=wv[d * 128:(d + 1) * 128, :])
            # xT arrives in column chunks, in the order the first attention
            # window consumes them.
            for blk in range(NBLK):
                for d in range(DT):
                    nc.sync.dma_start(
                        out=xt_sb[d][:, blk * 512:(blk + 1) * 512],
                        in_=xT[d * 128:(d + 1) * 128, blk * 512:(blk + 1) * 512],
                    )
            nc.vector.memset(ones_sb, 1.0)
            nc.vector.memset(zrow_sb, 0.0)

            qk_done = set()
            v_done = set()

            def ensure_qk(j, blk):
                # qkT_sb[j][:, blk] = (wqk[:, j] block).T @ xT[:, blk] + bias_j
                if (j, blk) in qk_done:
                    return
                qk_done.add((j, blk))
                pt = pp.tile([DH, 512], F32, tag="pj", name="pt", bufs=1)
                for d in range(DT):
                    nc.tensor.matmul(
                        pt,
                        lhsT=wqk_sb[d][:, j * DH:(j + 1) * DH],
                        rhs=xt_sb[d][:, blk * 512:(blk + 1) * 512],
                        start=(d == 0),
                        stop=(d == DT - 1),
                    )
                nc.vector.tensor_scalar_add(
                    out=qkT_sb[j][:, blk * 512:(blk + 1) * 512],
                    in0=pt,
                    scalar1=bqk_sb[:, j:j + 1],
                )

            def ensure_v(t):
                if t in v_done:
                    return
                v_done.add(t)
                pv = pp.tile([128, VW], F32, tag="pj", name="pv", bufs=1)
                for d in range(DT):
                    nc.tensor.matmul(
                        pv,
                        lhsT=xt_sb[d][:, t * 128:(t + 1) * 128],
                        rhs=wv_sb[d],
                        start=(d == 0),
                        stop=False,
                    )
                nc.tensor.matmul(
                    pv,
                    lhsT=ones_sb[:, t * 128:(t + 1) * 128],
                    rhs=wvaug_sb,
                    start=False,
                    stop=True,
                )
                nc.vector.tensor_copy(out=v_sb[:, t * VW:(t + 1) * VW], in_=pv)

            # filler: projection units to slip into PE slack inside the
            # ACT-bound attention stream, ordered by deadline.
            filler = []
            for b in range(1, NBLK):
                filler.append((0, b))       # q_h0 blk b: before window (0, b)
                filler.append((3, b - 1))   # k_h1: all before head 1
            filler.append((3, NBLK - 1))
            for b in range(NBLK):
                filler.append((1, b))       # q_h1 blk b: before window (1, b)
            fill_state = {"i": 0, "tick": 0}

            def pop_filler():
                fill_state["tick"] += 1
                if fill_state["tick"] % 4 == 0 and fill_state["i"] < len(filler):
                    j, b = filler[fill_state["i"]]
                    fill_state["i"] += 1
                    ensure_qk(j, b)

            def attn_nw(h, nw):
                qT = qkT_sb[h]
                kT = qkT_sb[2 + h]
                ensure_qk(h, nw)
                pva = pp.tile([128, 512], F32, tag="pva", name="pva", bufs=1)
                # Zero the accumulator bank with a K=1 matmul so every PV
                # matmul can be a plain accumulate (order-independent).
                nc.tensor.matmul(
                    pva,
                    lhsT=ones_sb[:, :128],
                    rhs=zrow_sb,
                    start=True,
                    stop=True,
                )

                def emit_pv(g0, gsz, ex):
                    if variant == "nopv":
                        return
                    for i in range(gsz):
                        mt = g0 + i
                        for ns in range(4):
                            nc.tensor.matmul(
                                pva[:, ns * 97:ns * 97 + 97],
                                lhsT=ex[:, i * 512 + ns * 128:i * 512 + (ns + 1) * 128],
                                rhs=v_sb[:, mt * VW + h * 97:mt * VW + h * 97 + 97],
                                start=False,
                                stop=(mt == NT - 1 and ns == 3),
                                skip_group_check=True,
                            )

                pending = None  # software pipeline: PV(g-1) after scores(g)
                for (g0, gsz) in groups:
                    for b in range((g0 + gsz - 1) * 128 // 512 + 1):
                        ensure_qk(2 + h, b)
                    for t in range(g0, g0 + gsz):
                        ensure_v(t)
                    sc = pp.tile([128, 512 * gsz], F32, tag="sc", name="sc")
                    for i in range(gsz):
                        mt = g0 + i
                        nc.tensor.matmul(
                            sc[:, i * 512:(i + 1) * 512],
                            lhsT=kT[:, mt * 128:(mt + 1) * 128],
                            rhs=qT[:, nw * 512:(nw + 1) * 512],
                            start=True,
                            stop=True,
                        )
                    ex = work.tile([128, 512 * gsz], F16, tag="ex", name="ex", bufs=4)
                    if variant == "noexp":
                        # timing ablation: near-free ACT op keeps deps intact
                        nc.scalar.activation(out=ex[:, :8], in_=sc[:, :8], func=Exp)
                    else:
                        nc.scalar.activation(out=ex, in_=sc, func=Exp)
                    if pending is not None:
                        emit_pv(*pending)
                        pop_filler()
                    pending = (g0, gsz, ex)
                emit_pv(*pending)
                pop_filler()
                rec = work.tile([128, 4], F32, tag="rec", name="rec", bufs=2)
                nc.vector.reciprocal(
                    out=rec,
                    in_=pva[:, :4 * 97].rearrange("p (a b) -> p a b", b=97)[:, :, DH],
                )
                ob = work.tile([128, 4 * DH], F32, tag="ob", name="ob")
                for ns in range(4):
                    nc.vector.tensor_scalar_mul(
                        out=ob[:, ns * DH:(ns + 1) * DH],
                        in0=pva[:, ns * 97:ns * 97 + DH],
                        scalar1=rec[:, ns:ns + 1],
                    )
                nc.sync.dma_start(
                    out=out[h, nw * 512:(nw + 1) * 512, :].rearrange(
                        "(a p) c -> p a c", p=128
                    ),
                    in_=ob.rearrange("p (a c) -> p a c", c=DH),
                )

            # Emission order tuned for overlap: head-0 q/k projection and V
            # first, then attention for head 0 with head-1 projections
            # slipped in between the first windows.
            def body(_i=None):
                qk_done.clear()
                v_done.clear()
                fill_state["i"] = 0
                fill_state["tick"] = 0
                for h in range(2):
                    for nw in range(NBLK):
                        attn_nw(h, nw)
                # backstop: anything the filler didn't reach
                for j, b in filler:
                    ensure_qk(j, b)

            if loop_iters == 1:
                body()
            else:
                with tc.For_i(0, loop_iters, 1) as _i:
                    body(_i)

    nc.compile()
    return nc


def get_program(loop_iters=1, variant="full"):
    key = ("nc", loop_iters, variant)
    if key not in _CACHE:
        _CACHE[key] = build_program(loop_iters, variant)
    return _CACHE[key]


def make_in_maps(x, W_qkv, b_qkv):
    x = np.asarray(x, np.float32)
    W = np.asarray(W_qkv, np.float32)
    b = np.asarray(b_qkv, np.float32)
    Wq, Wk, Wv = W[:, :DIM], W[:, DIM:2 * DIM], W[:, 2 * DIM:]
    bq, bk, bv = b[:DIM], b[DIM:2 * DIM], b[2 * DIM:]

    in_maps = []
    for c in range(NCORES):
        bb, hp = divmod(c, 4)
        h0 = 2 * hp
        s = slice(h0 * DH, (h0 + 1) * DH)
        s1 = slice((h0 + 1) * DH, (h0 + 2) * DH)
        xT = np.ascontiguousarray(x[bb].T).astype(np.float16)
        wqk = np.concatenate(
            [Wq[:, s] * SCALE, Wq[:, s1] * SCALE, Wk[:, s], Wk[:, s1]], axis=1
        ).astype(np.float16)
        bqk = np.stack(
            [bq[s] * SCALE, bq[s1] * SCALE, bk[s], bk[s1]], axis=1
        ).astype(np.float32)
        wv = np.zeros((DIM, VW), np.float16)
        wv[:, 0:DH] = Wv[:, s].astype(np.float16)
        wv[:, DH + 1:2 * DH + 1] = Wv[:, s1].astype(np.float16)
        wvaug = np.zeros((1, VW), np.float16)
        wvaug[0, 0:DH] = bv[s].astype(np.float16)
        wvaug[0, DH] = 1.0
        wvaug[0, DH + 1:2 * DH + 1] = bv[s1].astype(np.float16)
        wvaug[0, 2 * DH + 1] = 1.0
        in_maps.append(
            {"xT": xT, "wqk": wqk, "bqk": bqk, "wv": wv, "wvaug": wvaug}
        )
    return in_maps


def gather_out(results):
    out = np.empty((B, N, DIM), np.float32)
    for c in range(NCORES):
        bb, hp = divmod(c, 4)
        o = np.asarray(results[c]["out"], np.float32)  # [2, N, DH]
        out[bb, :, (2 * hp) * DH:(2 * hp + 1) * DH] = o[0]
        out[bb, :, (2 * hp + 1) * DH:(2 * hp + 2) * DH] = o[1]
    return out


def run(x, W_qkv, b_qkv, trace=False, **kw):
    from concourse.bass_utils import run_bass_kernel_spmd

    nc = get_program()
    in_maps = make_in_maps(x, W_qkv, b_qkv)
    res = run_bass_kernel_spmd(nc, in_maps, list(range(NCORES)), trace=trace, **kw)
    return gather_out(res.results), res


def kernel(x, W_qkv, b_qkv):
    out, _ = run(x, W_qkv, b_qkv)
    return out
